# revision 10
# baseline (speedup 1.0000x reference)
# Trainium2 Bass kernel for single-head causal attention
#   q = x@Wq, k = x@Wk, v = x@Wv   (x [B,T,C], W* [C,H])
#   out = softmax(mask(q k^T / sqrt(C))) @ v
# B=512, T=142, C=512, H=64.  Data-parallel over B across 8 NeuronCores.
#
# Device-side layout strategy (per core, 64 batches = 9088 tokens):
#  - host feeds x^T  [4,128,9088]  (contraction dim C on partitions)
#  - qT = Wq-stationary matmuls -> psum [128,*] rows 0:64 (Wq zero-padded)
#  - k,v packed:  [Wk|Wv] stationary -> psum rows 0:64 = kT, 64:128 = vT
#  - scores weiT[s,t] = kT-stationary matmul; causal mask added via one
#    identity-stationary matmul accumulating a mask tile into PSUM
#  - exp on ScalarE (scale=C^-0.5 fused), result bf16 in SBUF
#  - v natural [s,h] via identity-matmul transpose of vT
#  - AV: exp-scores stationary, rhs = [v | ones] -> out [t, 65] where
#    col 64 = softmax denominator; division happens on host (glue).
# Groups of 3 batches; group PAIRS share one x DMA and one output DMA
# to keep the SP sequencer / HWDGE ring off the critical path.
import os

import numpy as np
import ml_dtypes

B, T, C, H = 512, 142, 512, 64
NCORES = 8
NB = B // NCORES            # 64 batches per core
NT = NB * T                 # 9088 tokens per core
GB = 3                      # batches per processing group
NG = (NB + GB - 1) // GB    # 22 groups (21 full + 1 single)
SCALE = float(C) ** -0.5
NEG = -1e30
TW = 65                     # out block width: H + 1 denominator column

_CACHE = {}


def _groups():
    return [(g * GB, min(GB, NB - g * GB)) for g in range(NG)]


def _build_nc():
    import concourse.bacc as bacc
    import concourse.mybir as mybir
    from concourse.tile import TileContext

    fp32 = mybir.dt.float32
    bf16 = mybir.dt.bfloat16
    Exp = mybir.ActivationFunctionType.Exp

    nc = bacc.Bacc(
        "TRN2",
        target_bir_lowering=False,
        debug=False,
        enable_asserts=False,
        num_devices=NCORES,
    )

    xt = nc.dram_tensor("xt", [4, 128, NT], bf16, kind="ExternalInput").ap()
    # all 8 weight chunks in one tensor: [Wq|0] chunks then [Wk|Wv] chunks
    wts = nc.dram_tensor("wts", [8, 128, 128], bf16, kind="ExternalInput").ap()
    # constants blob: cols 0:426 mask3, 426:468 mskt3 (rows 0:14),
    # 468:596 identity128, 596:660 idhi
    cst = nc.dram_tensor("cst", [128, 660], bf16, kind="ExternalInput").ap()
    om = nc.dram_tensor("om", [NG, 128, GB * TW], fp32, kind="ExternalOutput").ap()
    ot = nc.dram_tensor("ot", [NG, 14, GB * TW], fp32, kind="ExternalOutput").ap()

    GT = GB * T           # 426 token columns per full group
    TAIL0 = GT            # col offset of tail score blocks in psc
    groups = _groups()
    pairs = [(2 * p, min(2, NG - 2 * p)) for p in range((NG + 1) // 2)]

    with TileContext(nc) as tc:
        with (
            tc.tile_pool(name="const", bufs=1) as cpool,
            tc.tile_pool(name="xtp", bufs=2) as xpool,
            tc.tile_pool(name="work", bufs=3) as wpool,
            tc.tile_pool(name="psum", bufs=1, space="PSUM") as ppool,
        ):
            wts_sb = cpool.tile([128, 8 * 128], bf16)
            cst_sb = cpool.tile([128, 660], bf16)
            nc.sync.dma_start(
                out=wts_sb.rearrange("p (c w) -> p c w", c=8),
                in_=wts.rearrange("c p w -> p c w"))
            nc.sync.dma_start(out=cst_sb[:, :], in_=cst)

            def wq_c(c):
                return wts_sb[:, c * 128:(c + 1) * 128]

            def wkv_c(c):
                return wts_sb[:, 512 + c * 128:512 + (c + 1) * 128]

            msk3_sb = cst_sb[:, 0:426]
            mskt3_sb = cst_sb[0:14, 426:468]
            iden_sb = cst_sb[:, 468:596]
            idhi_sb = cst_sb[:, 596:660]

            for g0, np_ in pairs:
                pg = groups[g0:g0 + np_]
                gtp = sum(nb for _, nb in pg) * T
                t0 = pg[0][0] * T

                xt_t = xpool.tile([128, 4 * 2 * GT], bf16, tag="xt")
                if g0 == 0:
                    for c in range(4):
                        nc.sync.dma_start(
                            out=xt_t[:, c * gtp:(c + 1) * gtp],
                            in_=xt[c, :, t0:t0 + gtp],
                        )
                else:
                    nc.sync.dma_start(
                        out=xt_t[:, 0:4 * gtp].rearrange("p (c t) -> p c t", c=4),
                        in_=xt[:, :, t0:t0 + gtp].rearrange("c p t -> p c t"),
                    )

                o_sb = wpool.tile([128, 2 * GB * TW], fp32, tag="o")
                o2_sb = wpool.tile([14, 2 * GB * TW], fp32, tag="o2")

                for s, (b0, nb) in enumerate(pg):
                    gt = nb * T
                    off = (b0 * T) - t0          # token offset within pair tile

                    # ---- QKV projections ----
                    pq = ppool.tile([128, GT], fp32, tag="pq", bufs=2)
                    pkv = ppool.tile([128, GT], fp32, tag="pkv", bufs=2)
                    for c in range(4):
                        rhs = xt_t[:, c * gtp + off:c * gtp + off + gt]
                        nc.tensor.matmul(
                            pq[:, :gt], lhsT=wq_c(c), rhs=rhs,
                            start=(c == 0), stop=(c == 3),
                        )
                    for c in range(4):
                        rhs = xt_t[:, c * gtp + off:c * gtp + off + gt]
                        nc.tensor.matmul(
                            pkv[:, :gt], lhsT=wkv_c(c), rhs=rhs,
                            start=(c == 0), stop=(c == 3),
                        )
                    q_sb = wpool.tile([64, GT], bf16, tag="q")
                    kv_sb = wpool.tile([128, GT], bf16, tag="kv")
                    nc.vector.tensor_copy(q_sb[:, :gt], pq[0:64, :gt])
                    nc.scalar.copy(kv_sb[:, :gt], pkv[:, :gt])

                    # ---- scores weiT[s,t] + causal mask ----
                    psc = ppool.tile([128, GT + GB * 14], fp32, tag="psc", bufs=2)
                    for j in range(nb):
                        cl = j * T
                        nc.tensor.matmul(
                            psc[:, cl:cl + T],
                            lhsT=kv_sb[0:64, cl:cl + 128],
                            rhs=q_sb[0:64, cl:cl + T],
                            start=True, stop=False,
                        )
                        nc.tensor.matmul(
                            psc[:, cl:cl + T],
                            lhsT=iden_sb,
                            rhs=msk3_sb[:, 0:T],
                            start=False, stop=True,
                        )
                        tco = TAIL0 + j * 14
                        nc.tensor.matmul(
                            psc[0:14, tco:tco + 14],
                            lhsT=kv_sb[0:64, cl + 128:cl + T],
                            rhs=q_sb[0:64, cl + 128:cl + T],
                            start=True, stop=False,
                        )
                        nc.tensor.matmul(
                            psc[0:14, tco:tco + 14],
                            lhsT=iden_sb[0:14, 0:14],
                            rhs=mskt3_sb[:, 0:14],
                            start=False, stop=True,
                        )

                    exp_sb = wpool.tile([128, GT + GB * 14], bf16, tag="exp")
                    nc.scalar.activation(
                        exp_sb[:, 0:gt], psc[:, 0:gt], Exp, scale=SCALE)
                    nc.scalar.activation(
                        exp_sb[0:14, TAIL0:TAIL0 + nb * 14],
                        psc[0:14, TAIL0:TAIL0 + nb * 14],
                        Exp, scale=SCALE,
                    )

                    # ---- v natural via identity-matmul transpose ----
                    pvt = ppool.tile([128, GB * 128], fp32, tag="pvt")
                    for j in range(nb):
                        cl = j * T
                        nc.tensor.matmul(
                            pvt[:, j * 64:(j + 1) * 64],
                            lhsT=kv_sb[64:128, cl:cl + 128],
                            rhs=idhi_sb[64:128, :],
                            start=True, stop=True,
                        )
                        nc.tensor.matmul(
                            pvt[0:14, GB * 64 + j * 64:GB * 64 + (j + 1) * 64],
                            lhsT=kv_sb[64:128, cl + 128:cl + T],
                            rhs=idhi_sb[64:128, :],
                            start=True, stop=True,
                        )
                    vex_sb = wpool.tile([128, GB * TW], bf16, tag="vex")
                    vext_sb = wpool.tile([14, GB * TW], bf16, tag="vext")
                    nc.vector.tensor_copy(
                        vex_sb.rearrange("p (b h) -> p b h", h=TW)[:, 0:nb, 0:64],
                        pvt[:, 0:nb * 64].rearrange("p (b h) -> p b h", h=64),
                    )
                    nc.vector.tensor_copy(
                        vext_sb.rearrange("p (b h) -> p b h", h=TW)[:, 0:nb, 0:64],
                        pvt[0:14, GB * 64:GB * 64 + nb * 64].rearrange(
                            "p (b h) -> p b h", h=64),
                    )
                    nc.vector.memset(
                        vex_sb.rearrange("p (b h) -> p b h", h=TW)[:, 0:nb, 64:65],
                        1.0)
                    nc.vector.memset(
                        vext_sb.rearrange("p (b h) -> p b h", h=TW)[:, 0:nb, 64:65],
                        1.0)

                    # ---- AV: out[t,0:64] = sum_s P^T[s,t] v[s,:], col64=denom ----
                    pout = ppool.tile([128, 2 * GB * TW], fp32, tag="pout")
                    TL = GB * TW
                    for j in range(nb):
                        cl = j * T
                        nc.tensor.matmul(
                            pout[:, j * TW:(j + 1) * TW],
                            lhsT=exp_sb[:, cl:cl + 128],
                            rhs=vex_sb[:, j * TW:(j + 1) * TW],
                            start=True, stop=True,
                        )
                        nc.tensor.matmul(
                            pout[0:14, TL + j * TW:TL + (j + 1) * TW],
                            lhsT=exp_sb[:, cl + 128:cl + T],
                            rhs=vex_sb[:, j * TW:(j + 1) * TW],
                            start=True, stop=False,
                        )
                        nc.tensor.matmul(
                            pout[0:14, TL + j * TW:TL + (j + 1) * TW],
                            lhsT=exp_sb[0:14, TAIL0 + j * 14:TAIL0 + (j + 1) * 14],
                            rhs=vext_sb[0:14, j * TW:(j + 1) * TW],
                            start=False, stop=True,
                        )

                    oc = s * GB * TW
                    nc.scalar.copy(
                        o_sb[:, oc:oc + nb * TW], pout[:, 0:nb * TW])
                    nc.vector.tensor_copy(
                        o2_sb[0:14, oc:oc + nb * TW],
                        pout[0:14, TL:TL + nb * TW])

                # ---- batched output stores (one per pair per tensor) ----
                last_nb = pg[-1][1]
                if np_ == 2 and last_nb == GB:
                    nc.gpsimd.dma_start(
                        out=om[g0:g0 + 2].rearrange("g p c -> p g c"),
                        in_=o_sb.rearrange("p (g c) -> p g c", g=2),
                    )
                    nc.gpsimd.dma_start(
                        out=ot[g0:g0 + 2].rearrange("g p c -> p g c"),
                        in_=o2_sb.rearrange("p (g c) -> p g c", g=2),
                    )
                else:
                    for s, (b0, nb) in enumerate(pg):
                        oc = s * GB * TW
                        nc.gpsimd.dma_start(
                            out=om[g0 + s, :, 0:nb * TW],
                            in_=o_sb[:, oc:oc + nb * TW])
                        nc.gpsimd.dma_start(
                            out=ot[g0 + s, :, 0:nb * TW],
                            in_=o2_sb[0:14, oc:oc + nb * TW])

    nc.compile()
    return nc


def _prep_shared(Wq, Wk, Wv):
    bf16 = ml_dtypes.bfloat16
    wq_pad = np.concatenate([Wq, np.zeros((C, H), np.float32)], axis=1)
    wkv = np.concatenate([Wk, Wv], axis=1)
    wts_np = np.concatenate(
        [
            np.ascontiguousarray(wq_pad.reshape(4, 128, 128)),
            np.ascontiguousarray(wkv.reshape(4, 128, 128)),
        ],
        axis=0,
    ).astype(bf16)

    s = np.arange(128)[:, None]
    t = np.arange(T)[None, :]
    msk = np.where(s <= t, 0.0, NEG).astype(np.float32)
    i = np.arange(14)[:, None]
    j = np.arange(14)[None, :]
    mskt = np.where(i <= j, 0.0, NEG).astype(np.float32)
    idhi = np.zeros((128, 64), np.float32)
    idhi[64 + np.arange(64), np.arange(64)] = 1.0

    cst = np.zeros((128, 660), np.float32)
    cst[:, 0:426] = np.tile(msk, (1, 3))
    cst[0:14, 426:468] = np.tile(mskt, (1, 3))
    cst[:, 468:596] = np.eye(128, dtype=np.float32)
    cst[:, 596:660] = idhi
    return dict(wts=wts_np, cst=cst.astype(bf16))


def _prep_core_xt(x_core):
    # x_core [NB, T, C] fp32 -> [4, 128, NT] bf16 (x^T, C on partitions)
    xt = x_core.reshape(NT, C).T            # [C, NT] view
    xt = np.ascontiguousarray(xt).reshape(4, 128, NT)
    return xt.astype(ml_dtypes.bfloat16)


def _assemble_core(om_np, ot_np):
    # om [NG, 128, GB*TW], ot [NG, 14, GB*TW] -> [NB, T, H] normalized
    bm = om_np.reshape(NG, 128, GB, TW).transpose(0, 2, 1, 3).reshape(NG * GB, 128, TW)
    bt = ot_np.reshape(NG, 14, GB, TW).transpose(0, 2, 1, 3).reshape(NG * GB, 14, TW)
    bm = bm[:NB].astype(np.float32)
    bt = bt[:NB].astype(np.float32)
    full = np.concatenate([bm, bt], axis=1)         # [NB, 142, TW]
    return full[:, :, 0:H] / full[:, :, H:H + 1]


def kernel(**inputs):
    x = np.asarray(inputs["x"], dtype=np.float32)
    Wq = np.asarray(inputs["Wq"], dtype=np.float32)
    Wk = np.asarray(inputs["Wk"], dtype=np.float32)
    Wv = np.asarray(inputs["Wv"], dtype=np.float32)

    from concourse.bass_utils import run_bass_kernel_spmd

    if "nc" not in _CACHE:
        _CACHE["nc"] = _build_nc()
    nc = _CACHE["nc"]

    shared = _prep_shared(Wq, Wk, Wv)
    in_maps = []
    for core in range(NCORES):
        m = dict(shared)
        m["xt"] = _prep_core_xt(x[core * NB:(core + 1) * NB])
        in_maps.append(m)

    trace = bool(int(os.environ.get("TRN_KERNEL_TRACE", "0")))
    res = run_bass_kernel_spmd(
        nc, in_maps, core_ids=list(range(NCORES)), trace=trace,
    )
    _CACHE["last_result"] = res

    outs = []
    for core in range(NCORES):
        r = res.results[core]
        outs.append(_assemble_core(np.asarray(r["om"]), np.asarray(r["ot"])))
    return np.concatenate(outs, axis=0).astype(np.float32)


# revision 11
# speedup vs baseline: 1.0037x; 1.0037x over previous
# Trainium2 Bass kernel for single-head causal attention
#   q = x@Wq, k = x@Wk, v = x@Wv   (x [B,T,C], W* [C,H])
#   out = softmax(mask(q k^T / sqrt(C))) @ v
# B=512, T=142, C=512, H=64.  Data-parallel over B across 8 NeuronCores.
#
# Device-side layout strategy (per core, 64 batches = 9088 tokens):
#  - host feeds x^T  [4,128,9088]  (contraction dim C on partitions)
#  - qT = Wq-stationary matmuls -> psum [128,*] rows 0:64 (Wq zero-padded)
#  - k,v packed:  [Wk|Wv] stationary -> psum rows 0:64 = kT, 64:128 = vT
#  - scores weiT[s,t] = kT-stationary matmul; causal mask added via one
#    identity-stationary matmul accumulating a mask tile into PSUM
#  - exp on ScalarE (scale=C^-0.5 fused), result bf16 in SBUF
#  - v natural [s,h] via identity-matmul transpose of vT
#  - AV: exp-scores stationary, rhs = [v | ones] -> out [t, 65] where
#    col 64 = softmax denominator; division happens on host (glue).
# Groups of 3 batches; group PAIRS share one x DMA and one output DMA
# to keep the SP sequencer / HWDGE ring off the critical path.
import os

import numpy as np
import ml_dtypes

B, T, C, H = 512, 142, 512, 64
NCORES = 8
NB = B // NCORES            # 64 batches per core
NT = NB * T                 # 9088 tokens per core
GB = 3                      # batches per processing group
NG = (NB + GB - 1) // GB    # 22 groups (21 full + 1 single)
SCALE = float(C) ** -0.5
NEG = -1e30
TW = 65                     # out block width: H + 1 denominator column

_CACHE = {}


def _groups():
    return [(g * GB, min(GB, NB - g * GB)) for g in range(NG)]


def _build_nc():
    import concourse.bacc as bacc
    import concourse.mybir as mybir
    from concourse.tile import TileContext

    fp32 = mybir.dt.float32
    bf16 = mybir.dt.bfloat16
    Exp = mybir.ActivationFunctionType.Exp

    nc = bacc.Bacc(
        "TRN2",
        target_bir_lowering=False,
        debug=False,
        enable_asserts=False,
        num_devices=NCORES,
    )

    xt = nc.dram_tensor("xt", [4, 128, NT], bf16, kind="ExternalInput").ap()
    # all 8 weight chunks in one tensor: [Wq|0] chunks then [Wk|Wv] chunks
    wts = nc.dram_tensor("wts", [8, 128, 128], bf16, kind="ExternalInput").ap()
    # constants blob: cols 0:426 mask3, 426:468 mskt3 (rows 0:14),
    # 468:596 identity128, 596:660 idhi
    cst = nc.dram_tensor("cst", [128, 660], bf16, kind="ExternalInput").ap()
    om = nc.dram_tensor("om", [NG, 128, GB * TW], fp32, kind="ExternalOutput").ap()
    ot = nc.dram_tensor("ot", [NG, 14, GB * TW], fp32, kind="ExternalOutput").ap()

    GT = GB * T           # 426 token columns per full group
    TAIL0 = GT            # col offset of tail score blocks in psc
    groups = _groups()
    pairs = [(2 * p, min(2, NG - 2 * p)) for p in range((NG + 1) // 2)]

    with TileContext(nc) as tc:
        with (
            tc.tile_pool(name="const", bufs=1) as cpool,
            tc.tile_pool(name="xtp", bufs=2) as xpool,
            tc.tile_pool(name="work", bufs=3) as wpool,
            tc.tile_pool(name="psum", bufs=1, space="PSUM") as ppool,
        ):
            wts_sb = cpool.tile([128, 8 * 128], bf16)
            cst_sb = cpool.tile([128, 660], bf16)
            nc.sync.dma_start(
                out=wts_sb.rearrange("p (c w) -> p c w", c=8),
                in_=wts.rearrange("c p w -> p c w"))
            nc.sync.dma_start(out=cst_sb[:, :], in_=cst)

            def wq_c(c):
                return wts_sb[:, c * 128:(c + 1) * 128]

            def wkv_c(c):
                return wts_sb[:, 512 + c * 128:512 + (c + 1) * 128]

            msk3_sb = cst_sb[:, 0:426]
            mskt3_sb = cst_sb[0:14, 426:468]
            iden_sb = cst_sb[:, 468:596]
            idhi_sb = cst_sb[:, 596:660]

            for g0, np_ in pairs:
                pg = groups[g0:g0 + np_]
                gtp = sum(nb for _, nb in pg) * T
                t0 = pg[0][0] * T

                xt_t = xpool.tile([128, 4 * 2 * GT], bf16, tag="xt")
                if g0 == 0:
                    for c in range(4):
                        nc.sync.dma_start(
                            out=xt_t[:, c * gtp:(c + 1) * gtp],
                            in_=xt[c, :, t0:t0 + gtp],
                        )
                else:
                    nc.sync.dma_start(
                        out=xt_t[:, 0:4 * gtp].rearrange("p (c t) -> p c t", c=4),
                        in_=xt[:, :, t0:t0 + gtp].rearrange("c p t -> p c t"),
                    )

                o_sb = wpool.tile([128, 2 * GB * TW], fp32, tag="o")
                o2_sb = wpool.tile([14, 2 * GB * TW], fp32, tag="o2")

                for s, (b0, nb) in enumerate(pg):
                    gt = nb * T
                    off = (b0 * T) - t0          # token offset within pair tile

                    # ---- QKV projections ----
                    pq = ppool.tile([128, GT], fp32, tag="pq", bufs=2)
                    pkv = ppool.tile([128, GT], fp32, tag="pkv", bufs=2)
                    # kv first: the ACT kv-copy (scores' stationary operand)
                    # then overlaps the q matmuls on PE
                    for c in range(4):
                        rhs = xt_t[:, c * gtp + off:c * gtp + off + gt]
                        nc.tensor.matmul(
                            pkv[:, :gt], lhsT=wkv_c(c), rhs=rhs,
                            start=(c == 0), stop=(c == 3),
                        )
                    q_sb = wpool.tile([64, GT], bf16, tag="q")
                    kv_sb = wpool.tile([128, GT], bf16, tag="kv")
                    nc.scalar.copy(kv_sb[:, :gt], pkv[:, :gt])
                    for c in range(4):
                        rhs = xt_t[:, c * gtp + off:c * gtp + off + gt]
                        nc.tensor.matmul(
                            pq[:, :gt], lhsT=wq_c(c), rhs=rhs,
                            start=(c == 0), stop=(c == 3),
                        )
                    nc.vector.tensor_copy(q_sb[:, :gt], pq[0:64, :gt])

                    # ---- scores weiT[s,t] + causal mask ----
                    psc = ppool.tile([128, GT + GB * 14], fp32, tag="psc", bufs=2)
                    for j in range(nb):
                        cl = j * T
                        nc.tensor.matmul(
                            psc[:, cl:cl + T],
                            lhsT=kv_sb[0:64, cl:cl + 128],
                            rhs=q_sb[0:64, cl:cl + T],
                            start=True, stop=False,
                        )
                        nc.tensor.matmul(
                            psc[:, cl:cl + T],
                            lhsT=iden_sb,
                            rhs=msk3_sb[:, 0:T],
                            start=False, stop=True,
                        )
                        tco = TAIL0 + j * 14
                        nc.tensor.matmul(
                            psc[0:14, tco:tco + 14],
                            lhsT=kv_sb[0:64, cl + 128:cl + T],
                            rhs=q_sb[0:64, cl + 128:cl + T],
                            start=True, stop=False,
                        )
                        nc.tensor.matmul(
                            psc[0:14, tco:tco + 14],
                            lhsT=iden_sb[0:14, 0:14],
                            rhs=mskt3_sb[:, 0:14],
                            start=False, stop=True,
                        )

                    exp_sb = wpool.tile([128, GT + GB * 14], bf16, tag="exp")
                    nc.scalar.activation(
                        exp_sb[:, 0:gt], psc[:, 0:gt], Exp, scale=SCALE)
                    nc.scalar.activation(
                        exp_sb[0:14, TAIL0:TAIL0 + nb * 14],
                        psc[0:14, TAIL0:TAIL0 + nb * 14],
                        Exp, scale=SCALE,
                    )

                    # ---- v natural via identity-matmul transpose ----
                    pvt = ppool.tile([128, GB * 128], fp32, tag="pvt")
                    for j in range(nb):
                        cl = j * T
                        nc.tensor.matmul(
                            pvt[:, j * 64:(j + 1) * 64],
                            lhsT=kv_sb[64:128, cl:cl + 128],
                            rhs=idhi_sb[64:128, :],
                            start=True, stop=True,
                        )
                        nc.tensor.matmul(
                            pvt[0:14, GB * 64 + j * 64:GB * 64 + (j + 1) * 64],
                            lhsT=kv_sb[64:128, cl + 128:cl + T],
                            rhs=idhi_sb[64:128, :],
                            start=True, stop=True,
                        )
                    vex_sb = wpool.tile([128, GB * TW], bf16, tag="vex")
                    vext_sb = wpool.tile([14, GB * TW], bf16, tag="vext")
                    nc.vector.tensor_copy(
                        vex_sb.rearrange("p (b h) -> p b h", h=TW)[:, 0:nb, 0:64],
                        pvt[:, 0:nb * 64].rearrange("p (b h) -> p b h", h=64),
                    )
                    nc.vector.tensor_copy(
                        vext_sb.rearrange("p (b h) -> p b h", h=TW)[:, 0:nb, 0:64],
                        pvt[0:14, GB * 64:GB * 64 + nb * 64].rearrange(
                            "p (b h) -> p b h", h=64),
                    )
                    nc.vector.memset(
                        vex_sb.rearrange("p (b h) -> p b h", h=TW)[:, 0:nb, 64:65],
                        1.0)
                    nc.vector.memset(
                        vext_sb.rearrange("p (b h) -> p b h", h=TW)[:, 0:nb, 64:65],
                        1.0)

                    # ---- AV: out[t,0:64] = sum_s P^T[s,t] v[s,:], col64=denom ----
                    pout = ppool.tile([128, 2 * GB * TW], fp32, tag="pout")
                    TL = GB * TW
                    for j in range(nb):
                        cl = j * T
                        nc.tensor.matmul(
                            pout[:, j * TW:(j + 1) * TW],
                            lhsT=exp_sb[:, cl:cl + 128],
                            rhs=vex_sb[:, j * TW:(j + 1) * TW],
                            start=True, stop=True,
                        )
                        nc.tensor.matmul(
                            pout[0:14, TL + j * TW:TL + (j + 1) * TW],
                            lhsT=exp_sb[:, cl + 128:cl + T],
                            rhs=vex_sb[:, j * TW:(j + 1) * TW],
                            start=True, stop=False,
                        )
                        nc.tensor.matmul(
                            pout[0:14, TL + j * TW:TL + (j + 1) * TW],
                            lhsT=exp_sb[0:14, TAIL0 + j * 14:TAIL0 + (j + 1) * 14],
                            rhs=vext_sb[0:14, j * TW:(j + 1) * TW],
                            start=False, stop=True,
                        )

                    oc = s * GB * TW
                    nc.scalar.copy(
                        o_sb[:, oc:oc + nb * TW], pout[:, 0:nb * TW])
                    nc.vector.tensor_copy(
                        o2_sb[0:14, oc:oc + nb * TW],
                        pout[0:14, TL:TL + nb * TW])

                # ---- batched output stores (one per pair per tensor) ----
                last_nb = pg[-1][1]
                if np_ == 2 and last_nb == GB:
                    nc.gpsimd.dma_start(
                        out=om[g0:g0 + 2].rearrange("g p c -> p g c"),
                        in_=o_sb.rearrange("p (g c) -> p g c", g=2),
                    )
                    nc.gpsimd.dma_start(
                        out=ot[g0:g0 + 2].rearrange("g p c -> p g c"),
                        in_=o2_sb.rearrange("p (g c) -> p g c", g=2),
                    )
                else:
                    for s, (b0, nb) in enumerate(pg):
                        oc = s * GB * TW
                        nc.gpsimd.dma_start(
                            out=om[g0 + s, :, 0:nb * TW],
                            in_=o_sb[:, oc:oc + nb * TW])
                        nc.gpsimd.dma_start(
                            out=ot[g0 + s, :, 0:nb * TW],
                            in_=o2_sb[0:14, oc:oc + nb * TW])

    nc.compile()
    return nc


def _prep_shared(Wq, Wk, Wv):
    bf16 = ml_dtypes.bfloat16
    wq_pad = np.concatenate([Wq, np.zeros((C, H), np.float32)], axis=1)
    wkv = np.concatenate([Wk, Wv], axis=1)
    wts_np = np.concatenate(
        [
            np.ascontiguousarray(wq_pad.reshape(4, 128, 128)),
            np.ascontiguousarray(wkv.reshape(4, 128, 128)),
        ],
        axis=0,
    ).astype(bf16)

    s = np.arange(128)[:, None]
    t = np.arange(T)[None, :]
    msk = np.where(s <= t, 0.0, NEG).astype(np.float32)
    i = np.arange(14)[:, None]
    j = np.arange(14)[None, :]
    mskt = np.where(i <= j, 0.0, NEG).astype(np.float32)
    idhi = np.zeros((128, 64), np.float32)
    idhi[64 + np.arange(64), np.arange(64)] = 1.0

    cst = np.zeros((128, 660), np.float32)
    cst[:, 0:426] = np.tile(msk, (1, 3))
    cst[0:14, 426:468] = np.tile(mskt, (1, 3))
    cst[:, 468:596] = np.eye(128, dtype=np.float32)
    cst[:, 596:660] = idhi
    return dict(wts=wts_np, cst=cst.astype(bf16))


def _prep_core_xt(x_core):
    # x_core [NB, T, C] fp32 -> [4, 128, NT] bf16 (x^T, C on partitions)
    xt = x_core.reshape(NT, C).T            # [C, NT] view
    xt = np.ascontiguousarray(xt).reshape(4, 128, NT)
    return xt.astype(ml_dtypes.bfloat16)


def _assemble_core(om_np, ot_np):
    # om [NG, 128, GB*TW], ot [NG, 14, GB*TW] -> [NB, T, H] normalized
    bm = om_np.reshape(NG, 128, GB, TW).transpose(0, 2, 1, 3).reshape(NG * GB, 128, TW)
    bt = ot_np.reshape(NG, 14, GB, TW).transpose(0, 2, 1, 3).reshape(NG * GB, 14, TW)
    bm = bm[:NB].astype(np.float32)
    bt = bt[:NB].astype(np.float32)
    full = np.concatenate([bm, bt], axis=1)         # [NB, 142, TW]
    return full[:, :, 0:H] / full[:, :, H:H + 1]


def kernel(**inputs):
    x = np.asarray(inputs["x"], dtype=np.float32)
    Wq = np.asarray(inputs["Wq"], dtype=np.float32)
    Wk = np.asarray(inputs["Wk"], dtype=np.float32)
    Wv = np.asarray(inputs["Wv"], dtype=np.float32)

    from concourse.bass_utils import run_bass_kernel_spmd

    if "nc" not in _CACHE:
        _CACHE["nc"] = _build_nc()
    nc = _CACHE["nc"]

    shared = _prep_shared(Wq, Wk, Wv)
    in_maps = []
    for core in range(NCORES):
        m = dict(shared)
        m["xt"] = _prep_core_xt(x[core * NB:(core + 1) * NB])
        in_maps.append(m)

    trace = bool(int(os.environ.get("TRN_KERNEL_TRACE", "0")))
    res = run_bass_kernel_spmd(
        nc, in_maps, core_ids=list(range(NCORES)), trace=trace,
    )
    _CACHE["last_result"] = res

    outs = []
    for core in range(NCORES):
        r = res.results[core]
        outs.append(_assemble_core(np.asarray(r["om"]), np.asarray(r["ot"])))
    return np.concatenate(outs, axis=0).astype(np.float32)
